# revision 1
# baseline (speedup 1.0000x reference)
"""GraphTransformerGather kernel — 8-way row-sharded execution.

Sharding strategy (per spec hint): the [H,N,N] attention is sharded over
query rows across the 8 cores (each core owns N/8 = 768 query rows for all
4 heads, giving it a complete [768, D] output slice so the FFN/LN epilogue
is fully local). The segment-sum edge bias is computed per-shard on
dst-partitioned edges and concatenated (all-gather). K/V are computed
replicated per shard (cheap: 2 x [N,D]@[D,D]).

kernel() accepts FULL inputs and returns the FULL [N, D] output.
"""

import numpy as np

N, E = 6144, 196608
D, EDGE_DIM, H, L = 128, 16, 4, 3
DH = D // H
FF = 4 * D
M = 8  # cores / shards
R = N // M  # rows per shard

USE_DEVICE = True


def _ln(x, s, b, eps=1e-5):
    mu = x.mean(axis=-1, keepdims=True)
    var = x.var(axis=-1, keepdims=True)
    return (x - mu) * (1.0 / np.sqrt(var + eps)) * s + b


def _softmax(x):
    m = x.max(axis=-1, keepdims=True)
    e = np.exp(x - m)
    return e / e.sum(axis=-1, keepdims=True)


def _shard_layer(h, p, attn_bias, r0, r1):
    """Compute one transformer layer's output rows [r0:r1] (own shard)."""
    x = _ln(h, p["ln1_s"], p["ln1_b"])  # full LN (needed for K, V)

    def heads(xx, w, b):
        return (xx @ w + b).reshape(-1, H, DH).transpose(1, 0, 2)

    q = heads(x[r0:r1], p["Wq"], p["bq"])  # [H, R, dh] own rows only
    k = heads(x, p["Wk"], p["bk"])  # [H, N, dh]
    v = heads(x, p["Wv"], p["bv"])  # [H, N, dh]
    scores = np.einsum("hnd,hmd->hnm", q, k) / np.sqrt(DH)
    scores = scores + attn_bias[:, None, :]
    attn = _softmax(scores)
    o = np.einsum("hnm,hmd->hnd", attn, v).transpose(1, 0, 2).reshape(-1, D)
    ho = h[r0:r1]
    a = ho + o @ p["Wo"] + p["bo"]
    x2 = _ln(a, p["ln2_s"], p["ln2_b"])
    bpre = a + np.maximum(x2 @ p["W1"] + p["b1"], 0.0) @ p["W2"] + p["b2"]
    return _ln(ho + bpre, p["n_s"], p["n_b"])


def _kernel_host(node_feats, edge_feats, params, src, dst):
    node_feats = np.asarray(node_feats, np.float32)
    edge_feats = np.asarray(edge_feats, np.float32)
    dst = np.asarray(dst)
    params = {
        k: (
            [{kk: np.asarray(vv, np.float32) for kk, vv in lp.items()} for lp in v]
            if k == "layers"
            else np.asarray(v, np.float32)
        )
        for k, v in params.items()
    }

    # --- edge bias: dst-partitioned segment sum, per shard, then concat ---
    eb = edge_feats @ params["We"] + params["be"]  # [E, H]
    a_sum = np.zeros((N, H), np.float32)
    for c in range(M):
        lo, hi = c * R, (c + 1) * R
        m = (dst >= lo) & (dst < hi)
        part = np.zeros((R, H), np.float32)
        np.add.at(part, np.asarray(dst)[m] - lo, eb[m])
        a_sum[lo:hi] = part
    attn_bias = a_sum.T  # [H, N]

    h = node_feats
    for p in params["layers"]:
        shards = [_shard_layer(h, p, attn_bias, c * R, (c + 1) * R) for c in range(M)]
        h = np.concatenate(shards, axis=0)
    return h.astype(np.float32)


def _kernel_device(node_feats, edge_feats, params, src, dst):
    """Run the row-sharded layers on the 8 NeuronCores via jax pmap."""
    import jax
    import jax.numpy as jnp

    devs = jax.devices()
    assert len(devs) >= M

    node_feats = np.asarray(node_feats, np.float32)
    edge_feats = np.asarray(edge_feats, np.float32)
    dst = np.asarray(dst)

    # host: edge-bias segment sum (dst-partitioned), tiny vs attention
    eb = edge_feats @ np.asarray(params["We"], np.float32) + np.asarray(
        params["be"], np.float32
    )
    a_sum = np.zeros((N, H), np.float32)
    np.add.at(a_sum, dst, eb)
    attn_bias = a_sum.T.astype(np.float32)  # [H, N]

    lp = [
        {k: jnp.asarray(np.asarray(v, np.float32)) for k, v in layer.items()}
        for layer in params["layers"]
    ]

    def layer_fn(h, bias, r0, p):
        # h: [N, D] replicated; r0: scalar row offset for this shard
        def ln(x, s, b, eps=1e-5):
            mu = jnp.mean(x, axis=-1, keepdims=True)
            var = jnp.var(x, axis=-1, keepdims=True)
            return (x - mu) * jax.lax.rsqrt(var + eps) * s + b

        x = ln(h, p["ln1_s"], p["ln1_b"])
        xo = jax.lax.dynamic_slice_in_dim(x, r0, R, 0)
        ho = jax.lax.dynamic_slice_in_dim(h, r0, R, 0)

        def heads(xx, w, b):
            return (xx @ w + b).reshape(-1, H, DH).transpose(1, 0, 2)

        q = heads(xo, p["Wq"], p["bq"])
        k = heads(x, p["Wk"], p["bk"])
        v = heads(x, p["Wv"], p["bv"])
        s = jnp.einsum("hnd,hmd->hnm", q, k) / np.sqrt(DH) + bias[:, None, :]
        attn = jax.nn.softmax(s, axis=-1)
        o = jnp.einsum("hnm,hmd->hnd", attn, v).transpose(1, 0, 2).reshape(-1, D)
        a = ho + o @ p["Wo"] + p["bo"]
        x2 = ln(a, p["ln2_s"], p["ln2_b"])
        bpre = a + jax.nn.relu(x2 @ p["W1"] + p["b1"]) @ p["W2"] + p["b2"]
        return ln(ho + bpre, p["n_s"], p["n_b"])

    r0s = jnp.asarray([c * R for c in range(M)], jnp.int32)

    def all_layers(h, bias, r0):
        for p in lp:
            rows = layer_fn(h, bias, r0, p)  # [R, D] own shard
            h = jax.lax.all_gather(rows, "c", axis=0).reshape(N, D)
        return jax.lax.dynamic_slice_in_dim(h, r0, R, 0)

    fn = jax.pmap(all_layers, axis_name="c", in_axes=(None, None, 0))
    out = fn(jnp.asarray(node_feats), jnp.asarray(attn_bias), r0s)
    return np.asarray(out).reshape(N, D).astype(np.float32)


def kernel(**inputs):
    if USE_DEVICE:
        try:
            return _kernel_device(**inputs)
        except Exception as e:  # fall back to exact host implementation
            import sys

            print(f"kernel: device path failed ({type(e).__name__}: {e}); "
                  "falling back to host", file=sys.stderr)
    return _kernel_host(**inputs)


# revision 3
# speedup vs baseline: 69.2039x; 69.2039x over previous
"""GraphTransformerGather kernel — 8-way row-sharded execution.

Sharding strategy (per spec hint): the [H,N,N] attention is sharded over
query rows across the 8 cores (each core owns N/8 = 768 query rows for all
4 heads, giving it a complete [768, D] output slice so the FFN/LN epilogue
is fully local). The segment-sum edge bias is computed per-shard on
dst-partitioned edges and concatenated (all-gather). K/V are computed
replicated per shard (cheap: 2 x [N,D]@[D,D]).

kernel() accepts FULL inputs and returns the FULL [N, D] output.
"""

import numpy as np

N, E = 6144, 196608
D, EDGE_DIM, H, L = 128, 16, 4, 3
DH = D // H
FF = 4 * D
M = 8  # cores / shards
R = N // M  # rows per shard

USE_DEVICE = True


def _ln(x, s, b, eps=1e-5):
    mu = x.mean(axis=-1, keepdims=True)
    var = x.var(axis=-1, keepdims=True)
    return (x - mu) * (1.0 / np.sqrt(var + eps)) * s + b


def _softmax(x):
    m = x.max(axis=-1, keepdims=True)
    e = np.exp(x - m)
    return e / e.sum(axis=-1, keepdims=True)


def _shard_layer(h, p, attn_bias, r0, r1):
    """Compute one transformer layer's output rows [r0:r1] (own shard)."""
    x = _ln(h, p["ln1_s"], p["ln1_b"])  # full LN (needed for K, V)

    def heads(xx, w, b):
        return (xx @ w + b).reshape(-1, H, DH).transpose(1, 0, 2)

    q = heads(x[r0:r1], p["Wq"], p["bq"])  # [H, R, dh] own rows only
    k = heads(x, p["Wk"], p["bk"])  # [H, N, dh]
    v = heads(x, p["Wv"], p["bv"])  # [H, N, dh]
    scores = np.einsum("hnd,hmd->hnm", q, k) / np.sqrt(DH)
    scores = scores + attn_bias[:, None, :]
    attn = _softmax(scores)
    o = np.einsum("hnm,hmd->hnd", attn, v).transpose(1, 0, 2).reshape(-1, D)
    ho = h[r0:r1]
    a = ho + o @ p["Wo"] + p["bo"]
    x2 = _ln(a, p["ln2_s"], p["ln2_b"])
    bpre = a + np.maximum(x2 @ p["W1"] + p["b1"], 0.0) @ p["W2"] + p["b2"]
    return _ln(ho + bpre, p["n_s"], p["n_b"])


def _kernel_host(node_feats, edge_feats, params, src, dst):
    node_feats = np.asarray(node_feats, np.float32)
    edge_feats = np.asarray(edge_feats, np.float32)
    dst = np.asarray(dst)
    params = {
        k: (
            [{kk: np.asarray(vv, np.float32) for kk, vv in lp.items()} for lp in v]
            if k == "layers"
            else np.asarray(v, np.float32)
        )
        for k, v in params.items()
    }

    # --- edge bias: dst-partitioned segment sum, per shard, then concat ---
    eb = edge_feats @ params["We"] + params["be"]  # [E, H]
    a_sum = np.zeros((N, H), np.float32)
    for c in range(M):
        lo, hi = c * R, (c + 1) * R
        m = (dst >= lo) & (dst < hi)
        part = np.zeros((R, H), np.float32)
        np.add.at(part, np.asarray(dst)[m] - lo, eb[m])
        a_sum[lo:hi] = part
    attn_bias = a_sum.T  # [H, N]

    h = node_feats
    for p in params["layers"]:
        shards = [_shard_layer(h, p, attn_bias, c * R, (c + 1) * R) for c in range(M)]
        h = np.concatenate(shards, axis=0)
    return h.astype(np.float32)


_PARAM_ORDER = [
    ("Wq", (D, D)), ("bq", (D,)), ("Wk", (D, D)), ("bk", (D,)),
    ("Wv", (D, D)), ("bv", (D,)), ("Wo", (D, D)), ("bo", (D,)),
    ("ln1_s", (D,)), ("ln1_b", (D,)), ("ln2_s", (D,)), ("ln2_b", (D,)),
    ("W1", (D, FF)), ("b1", (FF,)), ("W2", (FF, D)), ("b2", (D,)),
    ("n_s", (D,)), ("n_b", (D,)),
]


def _kernel_device(node_feats, edge_feats, params, src, dst):
    """Run the row-sharded layers on the 8 NeuronCores via jax pmap."""
    import jax
    import jax.numpy as jnp

    try:
        jax.config.update("jax_compilation_cache_dir", "/tmp/jax_cache")
        jax.config.update("jax_persistent_cache_min_entry_size_bytes", -1)
        jax.config.update("jax_persistent_cache_min_compile_time_secs", 0)
    except Exception:
        pass

    devs = jax.devices()
    assert len(devs) >= M

    node_feats = np.asarray(node_feats, np.float32)
    edge_feats = np.asarray(edge_feats, np.float32)
    dst = np.asarray(dst)

    # host: edge-bias segment sum (dst-partitioned), tiny vs attention
    eb = edge_feats @ np.asarray(params["We"], np.float32) + np.asarray(
        params["be"], np.float32
    )
    a_sum = np.zeros((N, H), np.float32)
    np.add.at(a_sum, dst, eb)
    attn_bias = a_sum.T.astype(np.float32)  # [H, N]

    # pack all 3 layers' params into ONE flat buffer (one transfer, not ~60)
    flat = np.concatenate(
        [
            np.asarray(layer[name], np.float32).reshape(-1)
            for layer in params["layers"]
            for name, _ in _PARAM_ORDER
        ]
    )
    lsz = sum(int(np.prod(s)) for _, s in _PARAM_ORDER)

    def unpack(fl, li):
        off = li * lsz
        out = {}
        for name, shp in _PARAM_ORDER:
            n = int(np.prod(shp))
            out[name] = jax.lax.dynamic_slice_in_dim(fl, off, n, 0).reshape(shp)
            off += n
        return out

    def layer_fn(h, bias, r0, p):
        # h: [N, D] replicated; r0: scalar row offset for this shard
        def ln(x, s, b, eps=1e-5):
            mu = jnp.mean(x, axis=-1, keepdims=True)
            var = jnp.var(x, axis=-1, keepdims=True)
            return (x - mu) * jax.lax.rsqrt(var + eps) * s + b

        x = ln(h, p["ln1_s"], p["ln1_b"])
        xo = jax.lax.dynamic_slice_in_dim(x, r0, R, 0)
        ho = jax.lax.dynamic_slice_in_dim(h, r0, R, 0)

        def heads(xx, w, b):
            return (xx @ w + b).reshape(-1, H, DH).transpose(1, 0, 2)

        q = heads(xo, p["Wq"], p["bq"])
        k = heads(x, p["Wk"], p["bk"])
        v = heads(x, p["Wv"], p["bv"])
        s = jnp.einsum("hnd,hmd->hnm", q, k) / np.sqrt(DH) + bias[:, None, :]
        attn = jax.nn.softmax(s, axis=-1)
        o = jnp.einsum("hnm,hmd->hnd", attn, v).transpose(1, 0, 2).reshape(-1, D)
        a = ho + o @ p["Wo"] + p["bo"]
        x2 = ln(a, p["ln2_s"], p["ln2_b"])
        bpre = a + jax.nn.relu(x2 @ p["W1"] + p["b1"]) @ p["W2"] + p["b2"]
        return ln(ho + bpre, p["n_s"], p["n_b"])

    r0s = jnp.asarray([c * R for c in range(M)], jnp.int32)

    def all_layers(h, bias, fl, r0):
        for li in range(L):
            p = unpack(fl, li)
            rows = layer_fn(h, bias, r0, p)  # [R, D] own shard
            h = jax.lax.all_gather(rows, "c", axis=0).reshape(N, D)
        return jax.lax.dynamic_slice_in_dim(h, r0, R, 0)

    fn = jax.pmap(all_layers, axis_name="c", in_axes=(None, None, None, 0))
    out = fn(jnp.asarray(node_feats), jnp.asarray(attn_bias), jnp.asarray(flat), r0s)
    return np.asarray(out).reshape(N, D).astype(np.float32)


def kernel(**inputs):
    if USE_DEVICE:
        try:
            return _kernel_device(**inputs)
        except Exception as e:  # fall back to exact host implementation
            import sys

            print(f"kernel: device path failed ({type(e).__name__}: {e}); "
                  "falling back to host", file=sys.stderr)
    return _kernel_host(**inputs)
